# revision 36
# baseline (speedup 1.0000x reference)
"""DropBlock kernel for Trainium2, 8 NeuronCores, batch-sharded data parallel.

Reference computation (B,C,H,W = 128,64,56,56, block=5, gamma=0.02):
    mask    = (noise < gamma)                       # (B,C,52,52) corner drops
    dilated = maxpool5x5_full_pad(mask)             # (B,C,56,56)
    block_mask = 1 - dilated
    out = block_mask * x * (numel / sum(block_mask))

Kernel formulation (exact, mask-first):
    q = (noise >= 0.02f) in {0,1}   # exact f32 compare (ACT Sign+Relu with
                                    # the gamma subtract fused into the bias,
                                    # computed in f32 inside the ACT datapath)
    block_mask = AND over the 5x5 window of q = PRODUCT over the window
                 (on {0,1} values min == mult)
    count = sum(block_mask), AllReduce across 8 cores.

Each core: 16 batches x 64 ch = 1024 images -> 8 tiles of 128 images
(images on partitions, image pixels along the free dimension).  The 5-wide
separable window-product uses log-step shifts (3 tensor_tensor ops per
axis) on 1.0-padded buffers so no boundary special cases are needed.

Engine split (per tile): ACT produces q (Sign+Relu), casts x to bf16, and
accumulates the per-partition count of the pooled mask (Identity+accum);
DVE runs only the bf16 product-pool tensor_tensor chain (2x mode).  All
compute stays off GpSimd — its TensorTensor is software-emulated on real
silicon and an order of magnitude slower than the cost model's estimate.
The post-collective tail is tiny: per tile one 4x-mode mask*scale
tensor_scalar plus one 2x bf16 multiply into the cast x, stored as bf16
(two roundings, ~0.2% rms; widened to f32 on the host).  The
cross-partition count reduction+broadcast is one PE matmul against a
ones matrix.
"""

import sys

sys.path.insert(0, "/opt/trn_rl_repo")

import numpy as np

import concourse.bacc as bacc
import concourse.bass as bass
import concourse.tile as tile
import concourse.mybir as mybir
from concourse.bass_utils import run_bass_kernel_spmd

N_CORES = 8
B, C, H, W = 128, 64, 56, 56
BLK = 5
GAMMA = 0.02
NH, NW = H - (BLK - 1), W - (BLK - 1)  # 52, 52 noise dims
B_SH = B // N_CORES  # 16 batches per core
IMGS = B_SH * C  # 1024 images per core
P = 128  # partitions
NTILES = IMGS // P  # 8 tiles per core
NPIX = NH * NW  # 2704 noise pixels/image
OPIX = H * W  # 3136 out pixels/image
TROWS = NH + 2 * (BLK - 1)  # 60 rows in padded vertical buffer
TFLAT = TROWS * NW  # 3120
VPW = NW + 2 * (BLK - 1)  # 60 cols in padded horizontal buffer (4+52+4)
COUNT_M = float(B * C * H * W)  # 25690112.0

# Largest f32 strictly below 0.02f: noise > gamma_lo  <=>  noise >= 0.02f.
GAMMA_LO = float(np.nextafter(np.float32(GAMMA), np.float32(0)))

F32 = mybir.dt.float32
BF16 = mybir.dt.bfloat16
MULT = mybir.AluOpType.mult
ADD = mybir.AluOpType.add
IDENT = mybir.ActivationFunctionType.Identity
SIGN = mybir.ActivationFunctionType.Sign
RELU = mybir.ActivationFunctionType.Relu

X_BUFS = 2  # f32 x staging ring; each tile is cast to bf16 right away

_CACHE = {}


def _build(single_core=False, repeat=1, no_cc=False):
    """Build + compile the SPMD bass module once.

    single_core=True builds a collective-free variant (the per-core count is
    used directly as the global count) for cost-model simulation only.
    repeat>1 unrolls the whole pipeline k times (benchmarking only).
    no_cc=True skips the AllReduce on the 8-core build (timing probe only —
    results are wrong by the per-core/global count ratio).
    """
    nc = bacc.Bacc("TRN2", target_bir_lowering=False, debug=False,
                   num_devices=1 if single_core else N_CORES)
    noise_ap = nc.dram_tensor("noise", [IMGS, NPIX], F32,
                              kind="ExternalInput").ap()
    x_ap = nc.dram_tensor("x", [IMGS, OPIX], F32, kind="ExternalInput").ap()
    out_ap = nc.dram_tensor("out", [IMGS, OPIX], BF16,
                            kind="ExternalOutput").ap()

    with tile.TileContext(nc) as tc:
        with (
            tc.tile_pool(name="buf", bufs=1) as bp,
            tc.tile_pool(name="stats", bufs=1) as stats_pool,
            tc.tile_pool(name="ps", bufs=1, space="PSUM") as psum_pool,
            tc.tile_pool(name="dram", bufs=1, space="DRAM") as dram_pool,
        ):
            bufs = {
                "nraw": [bp.tile([P, NPIX], F32, name=f"nraw{s}")
                         for s in range(2)],
                "tb": [bp.tile([P, TFLAT], BF16, name=f"tb{s}")
                       for s in range(2)],
                "a": [bp.tile([P, (TROWS - 1) * NW], BF16, name="a0")],
                "bt": [bp.tile([P, (TROWS - 3) * NW], BF16, name="bt0")],
                "vp": [bp.tile([P, H * VPW], BF16, name="vp0")],
                "a2": [bp.tile([P, H * VPW], BF16, name="a20")],
                "b2": [bp.tile([P, H * VPW], BF16, name="b20")],
                "junk": [bp.tile([P, OPIX], BF16, name="junk0")],
                "x": [bp.tile([P, OPIX], F32, name=f"x{s}")
                      for s in range(X_BUFS)],
            }
            dmstore = bp.tile([P, NTILES * OPIX], BF16, name="dmstore")
            xbstore = bp.tile([P, NTILES * OPIX], BF16, name="xbstore")
            ones_sb = bp.tile([P, P], F32, name="ones")
            nc.gpsimd.memset(ones_sb[:], 1.0)
            partials = stats_pool.tile([P, NTILES], F32)
            gbias = stats_pool.tile([P, 1], F32)
            nc.vector.memset(gbias[:], -GAMMA_LO)
            # warm the ACT function tables on a 1-element tile during the
            # DMA lead-in; otherwise LoadActFuncSet (~1.3us) lands in front
            # of the first real op and stalls the pipeline.
            warm = stats_pool.tile([P, 1], F32)
            nc.scalar.activation(warm[:], gbias[:], IDENT, bias=gbias[:, 0:1])
            nc.scalar.activation(warm[:], warm[:], SIGN)
            nc.scalar.activation(warm[:], warm[:], RELU)

            # 1.0-pads written once; every later op touches only data rows.
            for tb in bufs["tb"]:
                nc.gpsimd.memset(tb[:, 0:(BLK - 1) * NW], 1.0)
                nc.gpsimd.memset(tb[:, (NH + BLK - 1) * NW:TFLAT], 1.0)
            a0 = bufs["a"][0]
            nc.gpsimd.memset(a0[:, 0:3 * NW], 1.0)
            nc.gpsimd.memset(a0[:, 56 * NW:(TROWS - 1) * NW], 1.0)
            bt0 = bufs["bt"][0]
            nc.gpsimd.memset(bt0[:, 0:NW], 1.0)
            # row 56 = product of pad rows = 1.0, set once
            nc.gpsimd.memset(bt0[:, 56 * NW:(TROWS - 3) * NW], 1.0)
            vp3 = bufs["vp"][0][:].rearrange("p (h w) -> p h w", w=VPW)
            nc.gpsimd.memset(vp3[:, :, 0:BLK - 1], 1.0)
            nc.gpsimd.memset(vp3[:, :, W:VPW], 1.0)

            for rep in range(repeat):
                _emit_once(nc, tc, noise_ap, x_ap, out_ap, bufs, dmstore,
                           xbstore, ones_sb, partials, gbias, stats_pool,
                           psum_pool, dram_pool, single_core or no_cc, rep)

    nc.compile()
    return nc


def _emit_once(nc, tc, noise_ap, x_ap, out_ap, bufs, dmstore, xbstore,
               ones_sb, partials, gbias, stats_pool, psum_pool, dram_pool,
               single_core, rep):
    nraws, tbs, xts = bufs["nraw"], bufs["tb"], bufs["x"]
    a, bt, vp, a2, b2 = (bufs["a"][0], bufs["bt"][0], bufs["vp"][0],
                         bufs["a2"][0], bufs["b2"][0])
    junk = bufs["junk"][0]
    vp3 = vp[:].rearrange("p (h w) -> p h w", w=VPW)

    def load_noise(t):
        nraw = nraws[t % 2]
        if t == 0:
            # halve the cold-start DMA latency: the first q half can begin
            # as soon as rows 0..25 have landed
            nc.sync.dma_start(nraw[:, 0:NPIX // 2],
                              noise_ap[bass.ts(t, P), 0:NPIX // 2])
            nc.sync.dma_start(nraw[:, NPIX // 2:NPIX],
                              noise_ap[bass.ts(t, P), NPIX // 2:NPIX])
        else:
            nc.sync.dma_start(nraw[:], noise_ap[bass.ts(t, P), :])

    def load_x(t):
        # same (sync) queue as the noise loads, issued after the noise, so
        # the compute-critical noise tiles are never stuck behind an x
        # transfer; the ring's WAR hazard throttles against the bf16 cast.
        nc.sync.dma_start(xts[t % X_BUFS][:], x_ap[bass.ts(t, P), :])

    load_noise(0)
    for t in range(NTILES):
        nraw = nraws[t % 2]
        tb = tbs[t % 2]
        dm = dmstore[:, t * OPIX:(t + 1) * OPIX]
        # next tile's noise is issued here, before anything that reads it;
        # this tile's x load is issued later in the body so the noise-side
        # queue-count waits never cover an x transfer.
        if t + 1 < NTILES:
            load_noise(t + 1)
            # next tile's q = (noise >= 0.02f) on ACT: Sign with the gamma
            # subtract fused into the bias (f32-exact inside ACT), then
            # Relu in place -> {0,1} bf16 in the tb data rows.
            tn = tbs[(t + 1) % 2]
            trows = tn[:, (BLK - 1) * NW:(NH + BLK - 1) * NW]
            nc.scalar.activation(trows, nraws[(t + 1) % 2][:], SIGN,
                                 bias=gbias[:, 0:1])
            nc.scalar.activation(trows, trows, RELU)

        if t == 0:
            # tile 0's q runs on DVE itself (TS 2x mode, f32->bf16), in two
            # halves chasing the two DMA halves; noise > gamma_lo is exactly
            # noise >= 0.02f.  T rows 4..29 come from noise rows 0..25.
            nc.vector.tensor_scalar(
                tb[:, (BLK - 1) * NW:30 * NW], nraw[:, 0:NPIX // 2],
                GAMMA_LO, None, mybir.AluOpType.is_gt)
            nc.vector.tensor_scalar(
                tb[:, 30 * NW:(NH + BLK - 1) * NW], nraw[:, NPIX // 2:NPIX],
                GAMMA_LO, None, mybir.AluOpType.is_gt)

        # vertical window product, log-step: windows of 2, 4, then 5.
        # A rows 0..2 and 56..58 are products of pad rows only (= 1.0,
        # pre-set); only the data-dependent rows 3..55 are computed.
        if t == 0:
            # A rows 3..28 need only T rows 3..29 (first q half)
            nc.vector.tensor_tensor(
                a[:, 3 * NW:29 * NW], tb[:, 3 * NW:29 * NW],
                tb[:, 4 * NW:30 * NW], MULT)
            nc.vector.tensor_tensor(
                a[:, 29 * NW:56 * NW], tb[:, 29 * NW:56 * NW],
                tb[:, 30 * NW:57 * NW], MULT)
        else:
            nc.vector.tensor_tensor(
                a[:, 3 * NW:56 * NW], tb[:, 3 * NW:56 * NW],
                tb[:, 4 * NW:57 * NW], MULT)
        nc.vector.tensor_tensor(
            bt[:, NW:56 * NW], a[:, NW:56 * NW],
            a[:, 3 * NW:58 * NW], MULT)
        # V[r] = B[r] * T[r+4], r in 0..55 -> into padded Vp cols 4..55
        bt3 = bt[:].rearrange("p (h w) -> p h w", w=NW)
        tb3 = tb[:].rearrange("p (h w) -> p h w", w=NW)
        nc.vector.tensor_tensor(
            vp3[:, :, BLK - 1:BLK - 1 + NW], bt3[:, 0:H, :],
            tb3[:, BLK - 1:TROWS, :], MULT)

        # horizontal window product, log-step on strided row APs so only
        # the needed columns are processed (a2 cols 0..57, b2 cols 0..55)
        a23 = a2[:].rearrange("p (h w) -> p h w", w=VPW)
        b23 = b2[:].rearrange("p (h w) -> p h w", w=VPW)
        dm3 = dm.rearrange("p (h w) -> p h w", w=W)
        nc.vector.tensor_tensor(
            a23[:, :, 0:58], vp3[:, :, 0:58], vp3[:, :, 1:59], MULT)
        nc.vector.tensor_tensor(
            b23[:, :, 0:W], a23[:, :, 0:W], a23[:, :, 2:58], MULT)
        nc.vector.tensor_tensor(
            dm3[:, :, :], b23[:, :, 0:W], vp3[:, :, BLK - 1:VPW], MULT)

        load_x(t)
        # x cast to bf16 on ACT, freeing the f32 staging slot for t+2
        nc.scalar.activation(xbstore[:, t * OPIX:(t + 1) * OPIX],
                             xts[t % X_BUFS][:], IDENT)
        # dm IS the block mask ({0,1} bf16): count it on ACT (Identity into
        # a junk buffer with fused per-partition accumulation).
        nc.scalar.activation(junk[:], dm, IDENT,
                             accum_out=partials[:, t:t + 1])

    # ------------- global count -> scale = M / count_ones -------------
    # partials[0:7] are reduced while tile 7 is still computing; only the
    # tiny add of tile 7's count sits on the critical chain after its
    # ACT accumulation.
    phead = stats_pool.tile([P, 1], F32, name=f"phead{rep}", tag="phead")
    nc.vector.tensor_reduce(phead[:], partials[:, 0:NTILES - 1],
                            mybir.AxisListType.X, ADD)
    ptot = stats_pool.tile([P, 1], F32, name=f"ptot{rep}", tag="ptot")
    nc.vector.tensor_tensor(ptot[:], phead[:],
                            partials[:, NTILES - 1:NTILES], ADD)
    # cross-partition reduce + broadcast in one idle-PE matmul:
    # psum[m, 0] = sum_p ones[p, m] * ptot[p, 0] = total, for every m.
    ptot_ps = psum_pool.tile([P, 1], F32, name=f"ptot_ps{rep}", tag="pps")
    nc.tensor.matmul(ptot_ps[:], ones_sb[:], ptot[:], start=True, stop=True)
    pall = stats_pool.tile([P, 1], F32, name=f"pall{rep}", tag="pall")
    nc.vector.tensor_copy(pall[:], ptot_ps[:])
    if single_core:
        tot_sb = pall
    else:
        cc_in = dram_pool.tile([P, 1], F32, name=f"cc_in{rep}", tag="cc_in")
        cc_out = dram_pool.tile([P, 1], F32, name=f"cc_out{rep}",
                                tag="cc_out")
        nc.sync.dma_start(cc_in[:], pall[:])
        nc.gpsimd.collective_compute(
            "AllReduce", ADD,
            replica_groups=[list(range(N_CORES))],
            ins=[cc_in.opt()], outs=[cc_out.opt()])
        tot_sb = stats_pool.tile([P, 1], F32, name=f"tot{rep}", tag="tot")
        nc.sync.dma_start(tot_sb[:], cc_out[:])
    recip = stats_pool.tile([P, 1], F32, name=f"recip{rep}", tag="recip")
    nc.vector.reciprocal(recip[:], tot_sb[:])
    scale_sb = stats_pool.tile([P, 1], F32, name=f"scale{rep}", tag="scale")
    nc.vector.tensor_scalar_mul(scale_sb[:], recip[:], COUNT_M)

    # ------- phase 2: out = xb * (mask * scale), store bf16 -------
    for t in range(NTILES):
        dm = dmstore[:, t * OPIX:(t + 1) * OPIX]
        xb = xbstore[:, t * OPIX:(t + 1) * OPIX]
        # scaled mask into a free work buffer (a2/b2 alternate); the DVE
        # 4x tensor_scalar and the ACT scale-multiply split the tiles to
        # balance the two engines' tails
        msc = (a2 if t % 2 == 0 else b2)[:, 0:OPIX]
        if t % 2 == 0:
            nc.vector.tensor_scalar(msc, dm, scale_sb[:, 0:1], None, MULT)
        else:
            nc.scalar.activation(msc, dm, IDENT, scale=scale_sb[:, 0:1])
        nc.vector.tensor_tensor(xb, xb, msc, MULT)
        # alternate stores across both HWDGE queues so the final drain is
        # paced by aggregate DMA BW, not one queue's serialization
        eng = nc.sync if t % 2 == 0 else nc.scalar
        eng.dma_start(out_ap[bass.ts(t, P), :], xb)


def _get_nc():
    if "nc" not in _CACHE:
        _CACHE["nc"] = _build()
    return _CACHE["nc"]


def kernel(x: np.ndarray, noise: np.ndarray) -> np.ndarray:
    x = np.asarray(x, dtype=np.float32)
    noise = np.asarray(noise, dtype=np.float32)
    assert x.shape == (B, C, H, W) and noise.shape == (B, C, NH, NW)
    nc = _get_nc()
    in_maps = []
    for i in range(N_CORES):
        xs = np.ascontiguousarray(x[i * B_SH:(i + 1) * B_SH]).reshape(
            IMGS, OPIX)
        ns = np.ascontiguousarray(noise[i * B_SH:(i + 1) * B_SH]).reshape(
            IMGS, NPIX)
        in_maps.append({"x": xs, "noise": ns})
    res = run_bass_kernel_spmd(nc, in_maps, list(range(N_CORES)))
    out = np.empty((B, C, H, W), dtype=np.float32)
    for i in range(N_CORES):
        out[i * B_SH:(i + 1) * B_SH] = np.asarray(
            res.results[i]["out"]).astype(np.float32).reshape(
                B_SH, C, H, W)
    return out


# revision 37
# speedup vs baseline: 3.3049x; 3.3049x over previous
"""DropBlock kernel for Trainium2, 8 NeuronCores, batch-sharded data parallel.

Reference computation (B,C,H,W = 128,64,56,56, block=5, gamma=0.02):
    mask    = (noise < gamma)                       # (B,C,52,52) corner drops
    dilated = maxpool5x5_full_pad(mask)             # (B,C,56,56)
    block_mask = 1 - dilated
    out = block_mask * x * (numel / sum(block_mask))

Kernel formulation (exact, mask-first):
    q = (noise >= 0.02f) in {0,1}   # exact f32 compare (ACT Sign+Relu with
                                    # the gamma subtract fused into the bias,
                                    # computed in f32 inside the ACT datapath)
    block_mask = AND over the 5x5 window of q = PRODUCT over the window
                 (on {0,1} values min == mult)
    count = sum(block_mask), AllReduce across 8 cores.

Each core: 16 batches x 64 ch = 1024 images -> 8 tiles of 128 images
(images on partitions, image pixels along the free dimension).  The 5-wide
separable window-product uses log-step shifts (3 tensor_tensor ops per
axis) on 1.0-padded buffers so no boundary special cases are needed.

Engine split (per tile): ACT produces q (Sign+Relu), casts x to bf16, and
accumulates the per-partition count of the pooled mask (Identity+accum);
DVE runs only the bf16 product-pool tensor_tensor chain (2x mode).  All
compute stays off GpSimd — its TensorTensor is software-emulated on real
silicon and an order of magnitude slower than the cost model's estimate.
The post-collective tail is tiny: per tile one 4x-mode mask*scale
tensor_scalar plus one 2x bf16 multiply into the cast x, stored as bf16
(two roundings, ~0.2% rms; widened to f32 on the host).  The
cross-partition count reduction+broadcast is one PE matmul against a
ones matrix.
"""

import sys

sys.path.insert(0, "/opt/trn_rl_repo")

import numpy as np

import concourse.bacc as bacc
import concourse.bass as bass
import concourse.tile as tile
import concourse.mybir as mybir
from concourse.bass_utils import run_bass_kernel_spmd

N_CORES = 8
B, C, H, W = 128, 64, 56, 56
BLK = 5
GAMMA = 0.02
NH, NW = H - (BLK - 1), W - (BLK - 1)  # 52, 52 noise dims
B_SH = B // N_CORES  # 16 batches per core
IMGS = B_SH * C  # 1024 images per core
P = 128  # partitions
NTILES = IMGS // P  # 8 tiles per core
NPIX = NH * NW  # 2704 noise pixels/image
OPIX = H * W  # 3136 out pixels/image
TROWS = NH + 2 * (BLK - 1)  # 60 rows in padded vertical buffer
TFLAT = TROWS * NW  # 3120
VPW = NW + 2 * (BLK - 1)  # 60 cols in padded horizontal buffer (4+52+4)
COUNT_M = float(B * C * H * W)  # 25690112.0

# Largest f32 strictly below 0.02f: noise > gamma_lo  <=>  noise >= 0.02f.
GAMMA_LO = float(np.nextafter(np.float32(GAMMA), np.float32(0)))

F32 = mybir.dt.float32
BF16 = mybir.dt.bfloat16
MULT = mybir.AluOpType.mult
ADD = mybir.AluOpType.add
IDENT = mybir.ActivationFunctionType.Identity
SIGN = mybir.ActivationFunctionType.Sign
RELU = mybir.ActivationFunctionType.Relu

X_BUFS = 2  # f32 x staging ring; each tile is cast to bf16 right away

_CACHE = {}


def _build(single_core=False, repeat=1, no_cc=False):
    """Build + compile the SPMD bass module once.

    single_core=True builds a collective-free variant (the per-core count is
    used directly as the global count) for cost-model simulation only.
    repeat>1 unrolls the whole pipeline k times (benchmarking only).
    no_cc=True skips the AllReduce on the 8-core build (timing probe only —
    results are wrong by the per-core/global count ratio).
    """
    nc = bacc.Bacc("TRN2", target_bir_lowering=False, debug=False,
                   num_devices=1 if single_core else N_CORES)
    noise_ap = nc.dram_tensor("noise", [IMGS, NPIX], F32,
                              kind="ExternalInput").ap()
    x_ap = nc.dram_tensor("x", [IMGS, OPIX], F32, kind="ExternalInput").ap()
    out_ap = nc.dram_tensor("out", [IMGS, OPIX], BF16,
                            kind="ExternalOutput").ap()

    with tile.TileContext(nc) as tc:
        with (
            tc.tile_pool(name="buf", bufs=1) as bp,
            tc.tile_pool(name="stats", bufs=1) as stats_pool,
            tc.tile_pool(name="ps", bufs=1, space="PSUM") as psum_pool,
            tc.tile_pool(name="dram", bufs=1, space="DRAM") as dram_pool,
        ):
            bufs = {
                "nraw": [bp.tile([P, NPIX], F32, name=f"nraw{s}")
                         for s in range(2)],
                "tb": [bp.tile([P, TFLAT], BF16, name=f"tb{s}")
                       for s in range(2)],
                "a": [bp.tile([P, (TROWS - 1) * NW], BF16, name="a0")],
                "bt": [bp.tile([P, (TROWS - 3) * NW], BF16, name="bt0")],
                "vp": [bp.tile([P, H * VPW], BF16, name="vp0")],
                "a2": [bp.tile([P, H * VPW], BF16, name="a20")],
                "b2": [bp.tile([P, H * VPW], BF16, name="b20")],
                "junk": [bp.tile([P, OPIX], BF16, name="junk0")],
                "x": [bp.tile([P, OPIX], F32, name=f"x{s}")
                      for s in range(X_BUFS)],
            }
            dmstore = bp.tile([P, NTILES * OPIX], BF16, name="dmstore")
            xbstore = bp.tile([P, NTILES * OPIX], BF16, name="xbstore")
            ones_sb = bp.tile([P, P], F32, name="ones")
            nc.gpsimd.memset(ones_sb[:], 1.0)
            ones_bf = bp.tile([P, P], BF16, name="onesbf")
            nc.gpsimd.memset(ones_bf[:], 1.0)
            partials = stats_pool.tile([P, NTILES], F32)
            gbias = stats_pool.tile([P, 1], F32)
            nc.vector.memset(gbias[:], -GAMMA_LO)
            # warm the ACT function tables on a 1-element tile during the
            # DMA lead-in; otherwise LoadActFuncSet (~1.3us) lands in front
            # of the first real op and stalls the pipeline.
            warm = stats_pool.tile([P, 1], F32)
            nc.scalar.activation(warm[:], gbias[:], IDENT, bias=gbias[:, 0:1])
            nc.scalar.activation(warm[:], warm[:], SIGN)
            nc.scalar.activation(warm[:], warm[:], RELU)

            # 1.0-pads written once; every later op touches only data rows.
            for tb in bufs["tb"]:
                nc.gpsimd.memset(tb[:, 0:(BLK - 1) * NW], 1.0)
                nc.gpsimd.memset(tb[:, (NH + BLK - 1) * NW:TFLAT], 1.0)
            a0 = bufs["a"][0]
            nc.gpsimd.memset(a0[:, 0:3 * NW], 1.0)
            nc.gpsimd.memset(a0[:, 56 * NW:(TROWS - 1) * NW], 1.0)
            bt0 = bufs["bt"][0]
            nc.gpsimd.memset(bt0[:, 0:NW], 1.0)
            # row 56 = product of pad rows = 1.0, set once
            nc.gpsimd.memset(bt0[:, 56 * NW:(TROWS - 3) * NW], 1.0)
            vp3 = bufs["vp"][0][:].rearrange("p (h w) -> p h w", w=VPW)
            nc.gpsimd.memset(vp3[:, :, 0:BLK - 1], 1.0)
            nc.gpsimd.memset(vp3[:, :, W:VPW], 1.0)

            for rep in range(repeat):
                _emit_once(nc, tc, noise_ap, x_ap, out_ap, bufs, dmstore,
                           xbstore, (ones_sb, ones_bf), partials, gbias,
                           stats_pool, psum_pool, dram_pool,
                           single_core or no_cc, rep)

    nc.compile()
    return nc


def _emit_once(nc, tc, noise_ap, x_ap, out_ap, bufs, dmstore, xbstore,
               ones_pair, partials, gbias, stats_pool, psum_pool, dram_pool,
               single_core, rep):
    ones_sb, ones_bf = ones_pair
    cnt_ps = psum_pool.tile([P, OPIX], F32, name=f"cnt_ps{rep}", tag="cps")
    nraws, tbs, xts = bufs["nraw"], bufs["tb"], bufs["x"]
    a, bt, vp, a2, b2 = (bufs["a"][0], bufs["bt"][0], bufs["vp"][0],
                         bufs["a2"][0], bufs["b2"][0])
    junk = bufs["junk"][0]
    vp3 = vp[:].rearrange("p (h w) -> p h w", w=VPW)

    def load_noise(t):
        nraw = nraws[t % 2]
        if t == 0:
            # halve the cold-start DMA latency: the first q half can begin
            # as soon as rows 0..25 have landed
            nc.sync.dma_start(nraw[:, 0:NPIX // 2],
                              noise_ap[bass.ts(t, P), 0:NPIX // 2])
            nc.sync.dma_start(nraw[:, NPIX // 2:NPIX],
                              noise_ap[bass.ts(t, P), NPIX // 2:NPIX])
        else:
            nc.sync.dma_start(nraw[:], noise_ap[bass.ts(t, P), :])

    def load_x(t):
        # same (sync) queue as the noise loads, issued after the noise, so
        # the compute-critical noise tiles are never stuck behind an x
        # transfer; the ring's WAR hazard throttles against the bf16 cast.
        nc.sync.dma_start(xts[t % X_BUFS][:], x_ap[bass.ts(t, P), :])

    load_noise(0)
    for t in range(NTILES):
        nraw = nraws[t % 2]
        tb = tbs[t % 2]
        dm = dmstore[:, t * OPIX:(t + 1) * OPIX]
        # next tile's noise is issued here, before anything that reads it;
        # this tile's x load is issued later in the body so the noise-side
        # queue-count waits never cover an x transfer.
        if t + 1 < NTILES:
            load_noise(t + 1)
            # next tile's q = (noise >= 0.02f) on ACT: Sign with the gamma
            # subtract fused into the bias (f32-exact inside ACT), then
            # Relu in place -> {0,1} bf16 in the tb data rows.
            tn = tbs[(t + 1) % 2]
            trows = tn[:, (BLK - 1) * NW:(NH + BLK - 1) * NW]
            nc.scalar.activation(trows, nraws[(t + 1) % 2][:], SIGN,
                                 bias=gbias[:, 0:1])
            nc.scalar.activation(trows, trows, RELU)

        if t == 0:
            # tile 0's q runs on DVE itself (TS 2x mode, f32->bf16), in two
            # halves chasing the two DMA halves; noise > gamma_lo is exactly
            # noise >= 0.02f.  T rows 4..29 come from noise rows 0..25.
            nc.vector.tensor_scalar(
                tb[:, (BLK - 1) * NW:30 * NW], nraw[:, 0:NPIX // 2],
                GAMMA_LO, None, mybir.AluOpType.is_gt)
            nc.vector.tensor_scalar(
                tb[:, 30 * NW:(NH + BLK - 1) * NW], nraw[:, NPIX // 2:NPIX],
                GAMMA_LO, None, mybir.AluOpType.is_gt)

        # vertical window product, log-step: windows of 2, 4, then 5.
        # A rows 0..2 and 56..58 are products of pad rows only (= 1.0,
        # pre-set); only the data-dependent rows 3..55 are computed.
        if t == 0:
            # A rows 3..28 need only T rows 3..29 (first q half)
            nc.vector.tensor_tensor(
                a[:, 3 * NW:29 * NW], tb[:, 3 * NW:29 * NW],
                tb[:, 4 * NW:30 * NW], MULT)
            nc.vector.tensor_tensor(
                a[:, 29 * NW:56 * NW], tb[:, 29 * NW:56 * NW],
                tb[:, 30 * NW:57 * NW], MULT)
        else:
            nc.vector.tensor_tensor(
                a[:, 3 * NW:56 * NW], tb[:, 3 * NW:56 * NW],
                tb[:, 4 * NW:57 * NW], MULT)
        nc.vector.tensor_tensor(
            bt[:, NW:56 * NW], a[:, NW:56 * NW],
            a[:, 3 * NW:58 * NW], MULT)
        # V[r] = B[r] * T[r+4], r in 0..55 -> into padded Vp cols 4..55
        bt3 = bt[:].rearrange("p (h w) -> p h w", w=NW)
        tb3 = tb[:].rearrange("p (h w) -> p h w", w=NW)
        nc.vector.tensor_tensor(
            vp3[:, :, BLK - 1:BLK - 1 + NW], bt3[:, 0:H, :],
            tb3[:, BLK - 1:TROWS, :], MULT)

        # horizontal window product, log-step on strided row APs so only
        # the needed columns are processed (a2 cols 0..57, b2 cols 0..55)
        a23 = a2[:].rearrange("p (h w) -> p h w", w=VPW)
        b23 = b2[:].rearrange("p (h w) -> p h w", w=VPW)
        dm3 = dm.rearrange("p (h w) -> p h w", w=W)
        nc.vector.tensor_tensor(
            a23[:, :, 0:58], vp3[:, :, 0:58], vp3[:, :, 1:59], MULT)
        nc.vector.tensor_tensor(
            b23[:, :, 0:W], a23[:, :, 0:W], a23[:, :, 2:58], MULT)
        nc.vector.tensor_tensor(
            dm3[:, :, :], b23[:, :, 0:W], vp3[:, :, BLK - 1:VPW], MULT)

        load_x(t)
        # x cast to bf16 on ACT, freeing the f32 staging slot for t+2
        nc.scalar.activation(xbstore[:, t * OPIX:(t + 1) * OPIX],
                             xts[t % X_BUFS][:], IDENT)
        # dm IS the block mask ({0,1} bf16).  Tiles 0..6 are counted on
        # the otherwise-idle PE: accumulating ones-matmuls reduce over the
        # partition axis into a broadcast [P, OPIX] PSUM row-sum; ACT then
        # collapses it once, during tile 7, whose own count stays on ACT
        # (it is the only one on the critical path).
        if t < NTILES - 1:
            for c in range(0, OPIX, 512):
                hi = min(c + 512, OPIX)
                nc.tensor.matmul(cnt_ps[:, c:hi], ones_bf[:], dm[:, c:hi],
                                 start=(t == 0), stop=(t == NTILES - 2))
        else:
            nc.scalar.activation(junk[:], dm, IDENT,
                                 accum_out=partials[:, t:t + 1])

    # ------------- global count -> scale = M / count_ones -------------
    # partials[0:7] are reduced while tile 7 is still computing; only the
    # tiny add of tile 7's count sits on the critical chain after its
    # ACT accumulation.
    phead = stats_pool.tile([P, 1], F32, name=f"phead{rep}", tag="phead")
    nc.scalar.activation(junk[:], cnt_ps[:], IDENT, accum_out=phead[:])
    # tile 7's per-partition count -> broadcast total via one PE matmul
    ptot_ps = psum_pool.tile([P, 1], F32, name=f"ptot_ps{rep}", tag="pps")
    nc.tensor.matmul(ptot_ps[:], ones_sb[:],
                     partials[:, NTILES - 1:NTILES], start=True, stop=True)
    pall = stats_pool.tile([P, 1], F32, name=f"pall{rep}", tag="pall")
    nc.vector.tensor_tensor(pall[:], phead[:], ptot_ps[:], ADD)
    if single_core:
        tot_sb = pall
    else:
        cc_in = dram_pool.tile([P, 1], F32, name=f"cc_in{rep}", tag="cc_in")
        cc_out = dram_pool.tile([P, 1], F32, name=f"cc_out{rep}",
                                tag="cc_out")
        nc.sync.dma_start(cc_in[:], pall[:])
        nc.gpsimd.collective_compute(
            "AllReduce", ADD,
            replica_groups=[list(range(N_CORES))],
            ins=[cc_in.opt()], outs=[cc_out.opt()])
        tot_sb = stats_pool.tile([P, 1], F32, name=f"tot{rep}", tag="tot")
        nc.sync.dma_start(tot_sb[:], cc_out[:])
    recip = stats_pool.tile([P, 1], F32, name=f"recip{rep}", tag="recip")
    nc.vector.reciprocal(recip[:], tot_sb[:])
    scale_sb = stats_pool.tile([P, 1], F32, name=f"scale{rep}", tag="scale")
    nc.vector.tensor_scalar_mul(scale_sb[:], recip[:], COUNT_M)

    # ------- phase 2: out = xb * (mask * scale), store bf16 -------
    for t in range(NTILES):
        dm = dmstore[:, t * OPIX:(t + 1) * OPIX]
        xb = xbstore[:, t * OPIX:(t + 1) * OPIX]
        # scaled mask into a free work buffer (a2/b2 alternate); the DVE
        # 4x tensor_scalar and the ACT scale-multiply split the tiles to
        # balance the two engines' tails
        msc = (a2 if t % 2 == 0 else b2)[:, 0:OPIX]
        if t % 2 == 0:
            nc.vector.tensor_scalar(msc, dm, scale_sb[:, 0:1], None, MULT)
        else:
            nc.scalar.activation(msc, dm, IDENT, scale=scale_sb[:, 0:1])
        nc.vector.tensor_tensor(xb, xb, msc, MULT)
        # alternate stores across both HWDGE queues so the final drain is
        # paced by aggregate DMA BW, not one queue's serialization
        eng = nc.sync if t % 2 == 0 else nc.scalar
        eng.dma_start(out_ap[bass.ts(t, P), :], xb)


def _get_nc():
    if "nc" not in _CACHE:
        _CACHE["nc"] = _build()
    return _CACHE["nc"]


def kernel(x: np.ndarray, noise: np.ndarray) -> np.ndarray:
    x = np.asarray(x, dtype=np.float32)
    noise = np.asarray(noise, dtype=np.float32)
    assert x.shape == (B, C, H, W) and noise.shape == (B, C, NH, NW)
    nc = _get_nc()
    in_maps = []
    for i in range(N_CORES):
        xs = np.ascontiguousarray(x[i * B_SH:(i + 1) * B_SH]).reshape(
            IMGS, OPIX)
        ns = np.ascontiguousarray(noise[i * B_SH:(i + 1) * B_SH]).reshape(
            IMGS, NPIX)
        in_maps.append({"x": xs, "noise": ns})
    res = run_bass_kernel_spmd(nc, in_maps, list(range(N_CORES)))
    out = np.empty((B, C, H, W), dtype=np.float32)
    for i in range(N_CORES):
        out[i * B_SH:(i + 1) * B_SH] = np.asarray(
            res.results[i]["out"]).astype(np.float32).reshape(
                B_SH, C, H, W)
    return out
